# revision 1
# baseline (speedup 1.0000x reference)
"""MLA self-attention block (eval mode) on 8 Trainium2 NeuronCores.

Sharding: tensor-parallel over heads (16 heads -> 2 per core), batch kept
whole per core.  The small d_latent KV projection is recomputed (replicated)
per core.  Each core computes a partial output through its two heads' slice
of w_o; the host sums the 8 partials.

Math (per core, heads h0=2c, h1=2c+1):
  xT            = x^T (transposed on the host, passed as input)
  kvT  [L,T]    = w_dkv^T @ xT         (accumulated over C chunks)
  qT_h [S,T]    = w_q[:,h]^T @ xT      (un-absorbed: (x@Wq)@Wuk == x@(Wq@Wuk))
  k_effT [S,T]  = w_uk_h^T @ kvT       (w_uk absorbed into KEYS: 4x fewer
                                        att FLOPs than the q_lat form, since
                                        the att contraction drops L=512->S=128)
  attT [s,q]    = k_effT^T-tile @ qT   (causal: only s <= q tiles, 1 MM each)
  probs         = exp(scale*attT) * tri_mask   (logits are tiny -> no max-sub)
  yT  [S,q]     = lhsT=v [s,S], rhs=probs [s,q], accumulated over s
  den [1,q]     = ones^T @ probs
  yn            = yT * (1/den broadcast via ones-outer-product matmul)
  out_partial   = yn^T @ w_o_rows (2 heads accumulated)

All matmuls run in float32r (full PE rate at N>=256, ~1.5e-4 rel err).
Output DMA goes through the (otherwise idle) GpSimd SWDGE queue so x-tile
prefetches on the Sync HWDGE queue are never stuck behind stores.
"""

import sys
import os

sys.path.insert(0, "/opt/trn_rl_repo")

import numpy as np
from contextlib import ExitStack

import concourse.bass as bass
import concourse.tile as tile
from concourse import bacc, mybir
from concourse import bass_utils

F32 = mybir.dt.float32
F32R = mybir.dt.float32r

B, T, C = 2, 2048, 2048
H, S, L = 16, 128, 512
NCORES = 8
HPC = H // NCORES  # 2 heads per core
NT = T // 512  # 4 t-chunks of 512
SCALE = float(1.0 / np.sqrt(np.float32(C)))

_CACHE = {}


def _build():
    nc = bacc.Bacc("TRN2", target_bir_lowering=False, debug=False, num_devices=NCORES)

    xt_ap = nc.dram_tensor("xT", [B, C, T], F32, kind="ExternalInput").ap()
    w_dkv = nc.dram_tensor("w_dkv", [C, L], F32, kind="ExternalInput").ap()
    w_q_sl = nc.dram_tensor("w_q_sl", [C, HPC * S], F32, kind="ExternalInput").ap()
    w_ukT_sl = nc.dram_tensor("w_ukT_sl", [L, HPC * S], F32, kind="ExternalInput").ap()
    w_uv_sl = nc.dram_tensor("w_uv_sl", [L, HPC * S], F32, kind="ExternalInput").ap()
    w_o_sl = nc.dram_tensor("w_o_sl", [HPC * S, C], F32, kind="ExternalInput").ap()
    tri_d = nc.dram_tensor("tri", [128, 128], F32, kind="ExternalInput").ap()
    onesc_d = nc.dram_tensor("ones_col", [128, 1], F32, kind="ExternalInput").ap()
    onesr_d = nc.dram_tensor("ones_row", [1, 128], F32, kind="ExternalInput").ap()
    out_ap = nc.dram_tensor("out", [B, T, C], F32, kind="ExternalOutput").ap()

    w_dkv_r = w_dkv.rearrange("(cc p) l -> p cc l", p=128).bitcast(F32R)
    w_q_r = w_q_sl.rearrange("(cc p) f -> p cc f", p=128).bitcast(F32R)

    with tile.TileContext(nc) as tc:
        with ExitStack() as ctx:
            wpool = ctx.enter_context(tc.tile_pool(name="w", bufs=1))
            pers = ctx.enter_context(tc.tile_pool(name="pers", bufs=1))
            sb2 = ctx.enter_context(tc.tile_pool(name="sb2", bufs=2))
            sb4 = ctx.enter_context(tc.tile_pool(name="sb4", bufs=4))
            sb6 = ctx.enter_context(tc.tile_pool(name="sb6", bufs=6))
            psA = ctx.enter_context(tc.tile_pool(name="psA", bufs=4, space="PSUM"))
            psB = ctx.enter_context(tc.tile_pool(name="psB", bufs=3, space="PSUM"))
            psC = ctx.enter_context(tc.tile_pool(name="psC", bufs=1, space="PSUM"))

            # ---- constants + phase-1 weights on the fast Sync queue,
            #      per-c-chunk so the first matmuls start after ~0.5 MB ----
            wdkv_t = []
            wq_t = []
            for cc in range(16):
                wd = wpool.tile([128, L], F32R, tag=f"wdkv{cc}", name=f"wdkv{cc}")
                nc.scalar.dma_start(wd[:], w_dkv_r[:, cc, :])
                wdkv_t.append(wd)
                wqc = wpool.tile([128, HPC * S], F32R, tag=f"wq{cc}", name=f"wq{cc}")
                nc.scalar.dma_start(wqc[:], w_q_r[:, cc, :])
                wq_t.append(wqc)

            # ---- later-phase weights on the GpSimd (SWDGE) queue ----
            wukT = wpool.tile([128, 4, HPC * S], F32R, tag="wukT", name="wukT")
            nc.gpsimd.dma_start(
                wukT[:],
                w_ukT_sl.rearrange("(lc p) f -> p lc f", p=128).bitcast(F32R),
            )
            wuv = wpool.tile([128, 4, HPC * S], F32R, tag="wuv", name="wuv")
            nc.gpsimd.dma_start(
                wuv[:], w_uv_sl.rearrange("(lc p) f -> p lc f", p=128).bitcast(F32R)
            )
            wo = wpool.tile([128, HPC, C], F32R, tag="wo", name="wo")
            nc.gpsimd.dma_start(
                wo[:], w_o_sl.rearrange("(h p) f -> p h f", p=128).bitcast(F32R)
            )
            tri = wpool.tile([128, 128], F32R, tag="tri", name="tri")
            nc.gpsimd.dma_start(tri[:], tri_d.bitcast(F32R))
            onesc = wpool.tile([128, 1], F32R, tag="onesc", name="onesc")
            nc.gpsimd.dma_start(onesc[:], onesc_d.bitcast(F32R))
            onesr = wpool.tile([1, 128], F32R, tag="onesr", name="onesr")
            nc.gpsimd.dma_start(onesr[:], onesr_d.bitcast(F32R))

            pending_out = []  # deferred output-projection work items

            def emit_out(item):
                bb, jj, yn_ = item
                tb = jj * 512
                for tt in range(4):
                    for ncx in range(4):
                        op = psB.tile([128, 512], F32, tag="acc2", name="op")
                        for h in range(HPC):
                            nc.tensor.matmul(
                                op[:],
                                yn_[h][:, tt * 128 : (tt + 1) * 128],
                                wo[:, h, ncx * 512 : (ncx + 1) * 512],
                                start=(h == 0),
                                stop=(h == HPC - 1),
                            )
                        osb = sb4.tile([128, 512], F32, tag="osb", name="osb")
                        nc.vector.tensor_copy(osb[:], op[:])
                        nc.gpsimd.dma_start(
                            out_ap[
                                bb,
                                tb + tt * 128 : tb + (tt + 1) * 128,
                                ncx * 512 : (ncx + 1) * 512,
                            ],
                            osb[:],
                        )

            for b in range(B):
                kvT = pers.tile([128, 4, T], F32R, tag="kvT", name="kvT")
                vsb = pers.tile([128, T // 128, HPC * S], F32R, tag="vsb", name="vsb")
                keff = [
                    pers.tile([128, T], F32R, tag=f"keff{h}", name=f"keff{h}")
                    for h in range(HPC)
                ]

                for j in range(NT):
                    t0 = j * 512

                    # ======== phase 1: xT, kvT, qT for this t-chunk ========
                    kvps = []
                    qps = []
                    xt_sb = [None] * 16

                    def do_load(cc):
                        xt = sb6.tile([128, 512], F32R, tag="xt", name="xt")
                        nc.sync.dma_start(
                            xt[:],
                            xt_ap[
                                b, cc * 128 : (cc + 1) * 128, t0 : t0 + 512
                            ].bitcast(F32R),
                        )
                        xt_sb[cc] = xt

                    def do_mms(cc):
                        xt = xt_sb[cc]
                        for lc in range(4):
                            nc.tensor.matmul(
                                kvps[lc][:],
                                wdkv_t[cc][:, lc * 128 : (lc + 1) * 128],
                                xt[:],
                                start=(cc == 0),
                                stop=(cc == 15),
                            )
                        for h in range(HPC):
                            nc.tensor.matmul(
                                qps[h][:],
                                wq_t[cc][:, h * S : (h + 1) * S],
                                xt[:],
                                start=(cc == 0),
                                stop=(cc == 15),
                            )

                    # transpose 3 chunks ahead; slot the deferred output
                    # projection of the previous t-chunk into the bubble
                    do_load(0)
                    do_load(1)
                    do_load(2)
                    kvps.extend(
                        psA.tile([128, 512], F32, tag="acc4", name=f"kvps{i}")
                        for i in range(4)
                    )
                    qps.extend(
                        psB.tile([128, 512], F32, tag="acc2", name=f"qps{i}")
                        for i in range(HPC)
                    )
                    for cc in range(3, 16):
                        do_load(cc)
                        do_mms(cc - 3)
                    for cc in range(13, 16):
                        do_mms(cc)

                    def emit_qt(h):
                        qt = sb2.tile([128, 512], F32R, tag="qT", name="qt")
                        nc.vector.tensor_copy(qt[:], qps[h][:])
                        return qt

                    def emit_keff(h):
                        # k_effT[:, chunk] = w_uk_h^T-absorbed keys [S, 512]
                        kp = psB.tile([128, 512], F32, tag="acc2", name="kp")
                        for lc in range(4):
                            nc.tensor.matmul(
                                kp[:],
                                wukT[:, lc, h * S : (h + 1) * S],
                                kvT[:, lc, t0 : t0 + 512],
                                start=(lc == 0),
                                stop=(lc == 3),
                            )
                        nc.vector.tensor_copy(keff[h][:, t0 : t0 + 512], kp[:])

                    # ======== attention (split so head-0's off-diagonal
                    # work hides the kv/v evacuation latency) ========
                    nst = 4 * j + 4

                    class AttState:
                        pass

                    def att_begin(h, qt):
                        st = AttState()
                        st.h = h
                        st.qt = qt
                        st.yps = psB.tile([128, 512], F32, tag="acc2", name="yps")
                        st.dps = psC.tile([1, 512], F32, tag="den", name="dps")
                        st.prev = None
                        return st

                    def y_den(st, item):
                        i, n0, ex = item
                        nc.tensor.matmul(
                            st.yps[:, n0:512],
                            vsb[:, i, st.h * S : (st.h + 1) * S],
                            ex[:, n0:512],
                            start=(i == 0),
                            stop=(i == nst - 1),
                        )
                        nc.tensor.matmul(
                            st.dps[:, n0:512],
                            onesc[:],
                            ex[:, n0:512],
                            start=(i == 0),
                            stop=(i == nst - 1),
                        )

                    def att_steps(st, i_lo, i_hi):
                        for i in range(i_lo, i_hi):
                            n0 = (i - 4 * j) * 128 if i >= 4 * j else 0
                            aps = psA.tile([128, 512], F32, tag="acc4", name="aps")
                            nc.tensor.matmul(
                                aps[:, n0:512],
                                keff[st.h][:, i * 128 : (i + 1) * 128],
                                st.qt[:, n0:512],
                                start=True,
                                stop=True,
                            )
                            ex = sb6.tile([128, 512], F32R, tag="exp", name="ex")
                            nc.scalar.activation(
                                ex[:, n0:512],
                                aps[:, n0:512],
                                mybir.ActivationFunctionType.Exp,
                                scale=SCALE,
                            )
                            if i >= 4 * j:
                                nc.vector.tensor_mul(
                                    ex[:, n0 : n0 + 128],
                                    ex[:, n0 : n0 + 128],
                                    tri[:],
                                )
                            if st.prev is not None:
                                y_den(st, st.prev)
                            st.prev = (i, n0, ex)

                    def att_finish(st):
                        y_den(st, st.prev)
                        rec32 = sb2.tile([1, 512], F32, tag="rec32", name="rec32")
                        nc.vector.reciprocal_approx_fast(rec32[:], st.dps[:])
                        rec = sb2.tile([1, 512], F32R, tag="rec", name="rec")
                        nc.vector.tensor_copy(rec[:], rec32[:])
                        bps = psC.tile([128, 512], F32, tag="den", name="bps")
                        nc.tensor.matmul(
                            bps[:], onesr[:], rec[:], start=True, stop=True
                        )
                        bcs = sb2.tile([128, 512], F32, tag="bcs", name="bcs")
                        nc.vector.tensor_copy(bcs[:], bps[:])
                        y = sb4.tile([128, 512], F32R, tag="yn", name="y")
                        with nc.allow_low_precision(reason="f32r is fp32-width"):
                            nc.vector.tensor_mul(y[:], st.yps[:], bcs[:])
                        return y

                    # evacuate this chunk's kvT (frees acc4) and qT (frees
                    # acc2); the deferred output projection of the previous
                    # chunk fills the PE while DVE/ACT evacuate
                    for lc in range(4):
                        nc.vector.tensor_copy(kvT[:, lc, t0 : t0 + 512], kvps[lc][:])
                    qt0 = emit_qt(0)
                    qt1 = emit_qt(1)
                    if pending_out:
                        emit_out(pending_out.pop())

                    # head 0 off-diagonal attention touches only keff/vsb
                    # rows of previous chunks -> starts immediately
                    st0 = att_begin(0, qt0)
                    att_steps(st0, 0, 4 * j)
                    emit_keff(0)

                    # this chunk's v rows
                    for tt in range(4):
                        vp = psB.tile([128, HPC * S], F32, tag="acc2", name="vp")
                        for lc in range(4):
                            nc.tensor.matmul(
                                vp[:],
                                kvT[:, lc, t0 + tt * 128 : t0 + (tt + 1) * 128],
                                wuv[:, lc, :],
                                start=(lc == 0),
                                stop=(lc == 3),
                            )
                        nc.vector.tensor_copy(vsb[:, 4 * j + tt, :], vp[:])
                    emit_keff(1)

                    # head 0 diagonal + finish, then head 1 in full
                    att_steps(st0, 4 * j, nst)
                    y0 = att_finish(st0)
                    st1 = att_begin(1, qt1)
                    att_steps(st1, 0, nst)
                    y1 = att_finish(st1)
                    yn = [y0, y1]

                    pending_out.append((b, j, yn))

            emit_out(pending_out.pop())

    nc.compile()
    return nc


def _get_nc():
    if "nc" not in _CACHE:
        _CACHE["nc"] = _build()
    return _CACHE["nc"]


def kernel(x, w_dkv, w_uk, w_uv, w_q, w_o):
    x = np.asarray(x, dtype=np.float32)
    xT = np.ascontiguousarray(x.transpose(0, 2, 1))
    w_dkv = np.ascontiguousarray(np.asarray(w_dkv, dtype=np.float32))
    w_uk = np.ascontiguousarray(np.asarray(w_uk, dtype=np.float32))
    w_uv = np.ascontiguousarray(np.asarray(w_uv, dtype=np.float32))
    w_q = np.ascontiguousarray(np.asarray(w_q, dtype=np.float32))
    w_o = np.ascontiguousarray(np.asarray(w_o, dtype=np.float32))

    nc = _get_nc()

    tri = np.triu(np.ones((128, 128), dtype=np.float32))
    ones_col = np.ones((128, 1), dtype=np.float32)
    ones_row = np.ones((1, 128), dtype=np.float32)

    in_maps = []
    for c in range(NCORES):
        sl = slice(c * HPC * S, (c + 1) * HPC * S)
        in_maps.append(
            {
                "xT": xT,
                "w_dkv": w_dkv,
                "w_q_sl": np.ascontiguousarray(w_q[:, sl]),
                "w_ukT_sl": np.ascontiguousarray(w_uk[sl, :].T),
                "w_uv_sl": np.ascontiguousarray(w_uv[:, sl]),
                "w_o_sl": np.ascontiguousarray(w_o[sl, :]),
                "tri": tri,
                "ones_col": ones_col,
                "ones_row": ones_row,
            }
        )

    kwargs = dict(_CACHE.get("run_kwargs", {}))
    res = bass_utils.run_bass_kernel_spmd(
        nc, in_maps, core_ids=list(range(NCORES)), **kwargs
    )
    _CACHE["last_result"] = res

    acc = np.zeros((B, T, C), dtype=np.float64)
    for r in res.results:
        acc += r["out"]
    return acc.astype(np.float32)



# revision 3
# speedup vs baseline: 1.5196x; 1.5196x over previous
"""MLA self-attention block (eval mode) on 8 Trainium2 NeuronCores.

Sharding v2: batch x heads hybrid.  Core c handles batch b = c//4 and the
4 heads [4*(c%4), 4*(c%4)+4).  The d_latent KV projection is recomputed per
core but only for its own batch (half the replicated FLOPs of pure head-TP).
Each core computes a partial [T, C] output through its 4 heads' rows of w_o;
the host sums the 4 partials per batch.

Math (per core, heads h=0..3 local):
  xT            = x[b]^T (transposed on the host, bf16)
  kvT  [L,T]    = w_dkv^T @ xT         (accumulated over C chunks)
  qT_h [S,T]    = w_q[:,h]^T @ xT      (un-absorbed: (x@Wq)@Wuk == x@(Wq@Wuk))
  k_effT [S,T]  = w_uk_h^T @ kvT       (w_uk absorbed into KEYS: 4x fewer
                                        att FLOPs than the q_lat form)
  attT [s,q]    = k_effT^T-tile @ qT   (causal: only s <= q tiles)
  probs         = exp(scale*attT) * tri_mask   (logits are tiny -> no max-sub)
  yT  [S,q]     = lhsT=v [s,S], rhs=probs [s,q], accumulated over s
  den           = allones^T @ (pairwise-summed probs)  (broadcast rows, so no
                                        separate bcast matmul; pairing halves
                                        the PE cost of the denominator)
  yn            = yT * reciprocal(den)
  out_partial   = yn^T @ w_o_rows (4 heads accumulated)

All matmuls run in bf16 (same PE rate as f32r, half the SBUF/DMA, faster
LDWEIGHTS); accumulation stays fp32 in PSUM.  Phase 1 (kv+q) fills all 8
PSUM banks; the deferred output projection of the previous chunk and the
just-in-time v/keff matmuls run during the attention phase as PE filler
between exp-latency-bound attention steps.  Output DMA goes through the
GpSimd SWDGE queue so x-tile prefetches on the Sync HWDGE queue are never
stuck behind stores.
"""

import sys

sys.path.insert(0, "/opt/trn_rl_repo")

import numpy as np
import ml_dtypes
from contextlib import ExitStack

import concourse.bass as bass
import concourse.tile as tile
from concourse import bacc, mybir
from concourse import bass_utils

F32 = mybir.dt.float32
BF16 = mybir.dt.bfloat16

B, T, C = 2, 2048, 2048
H, S, L = 16, 128, 512
NCORES = 8
HPC = 4  # heads per core
NT = T // 512  # 4 t-chunks of 512
SCALE = float(1.0 / np.sqrt(np.float32(C)))

_CACHE = {}


def _build():
    nc = bacc.Bacc("TRN2", target_bir_lowering=False, debug=False, num_devices=NCORES)

    xt_ap = nc.dram_tensor("xT", [C, T], BF16, kind="ExternalInput").ap()
    w_dkv = nc.dram_tensor("w_dkv", [C, L], BF16, kind="ExternalInput").ap()
    w_q_sl = nc.dram_tensor("w_q_sl", [C, HPC * S], BF16, kind="ExternalInput").ap()
    w_ukT_sl = nc.dram_tensor("w_ukT_sl", [L, HPC * S], BF16, kind="ExternalInput").ap()
    w_uv_sl = nc.dram_tensor("w_uv_sl", [L, HPC * S], BF16, kind="ExternalInput").ap()
    w_o_sl = nc.dram_tensor("w_o_sl", [HPC * S, C], BF16, kind="ExternalInput").ap()
    tri_d = nc.dram_tensor("tri", [128, 128], BF16, kind="ExternalInput").ap()
    allones_d = nc.dram_tensor("allones", [128, 128], BF16, kind="ExternalInput").ap()
    out_ap = nc.dram_tensor("out", [T, C], F32, kind="ExternalOutput").ap()

    w_dkv_r = w_dkv.rearrange("(cc p) l -> p cc l", p=128)
    w_q_r = w_q_sl.rearrange("(cc p) f -> p cc f", p=128)

    with tile.TileContext(nc) as tc:
        with ExitStack() as ctx:
            wpool = ctx.enter_context(tc.tile_pool(name="w", bufs=1))
            pers = ctx.enter_context(tc.tile_pool(name="pers", bufs=1))
            sb2 = ctx.enter_context(tc.tile_pool(name="sb2", bufs=2))
            sb5 = ctx.enter_context(tc.tile_pool(name="sb5", bufs=5))
            sb6 = ctx.enter_context(tc.tile_pool(name="sb6", bufs=6))
            sbyn = ctx.enter_context(tc.tile_pool(name="sbyn", bufs=8))
            sbo = ctx.enter_context(tc.tile_pool(name="sbo", bufs=4))
            psA = ctx.enter_context(tc.tile_pool(name="psA", bufs=4, space="PSUM"))
            psB = ctx.enter_context(tc.tile_pool(name="psB", bufs=4, space="PSUM"))

            # ---- phase-1 weights per-c-chunk on the fast Sync queue so the
            #      first matmuls start after ~0.25 MB ----
            wdkv_t = []
            wq_t = []
            for cc in range(16):
                wd = wpool.tile([128, L], BF16, tag=f"wdkv{cc}", name=f"wdkv{cc}")
                nc.scalar.dma_start(wd[:], w_dkv_r[:, cc, :])
                wdkv_t.append(wd)
                wqc = wpool.tile([128, HPC * S], BF16, tag=f"wq{cc}", name=f"wq{cc}")
                nc.scalar.dma_start(wqc[:], w_q_r[:, cc, :])
                wq_t.append(wqc)

            # ---- later-phase weights on the GpSimd (SWDGE) queue ----
            wukT = wpool.tile([128, 4, HPC * S], BF16, tag="wukT", name="wukT")
            nc.gpsimd.dma_start(wukT[:], w_ukT_sl.rearrange("(lc p) f -> p lc f", p=128))
            wuv = wpool.tile([128, 4, HPC * S], BF16, tag="wuv", name="wuv")
            nc.gpsimd.dma_start(wuv[:], w_uv_sl.rearrange("(lc p) f -> p lc f", p=128))
            wo = wpool.tile([128, HPC, C], BF16, tag="wo", name="wo")
            nc.gpsimd.dma_start(wo[:], w_o_sl.rearrange("(h p) f -> p h f", p=128))
            tri = wpool.tile([128, 128], BF16, tag="tri", name="tri")
            nc.gpsimd.dma_start(tri[:], tri_d)
            allones = wpool.tile([128, 128], BF16, tag="allones", name="allones")
            nc.gpsimd.dma_start(allones[:], allones_d)

            # persistent per-batch state
            kvT = pers.tile([128, 4, T], BF16, tag="kvT", name="kvT")
            vsb = pers.tile([128, T // 128, HPC * S], BF16, tag="vsb", name="vsb")
            keff = [
                pers.tile([128, T], BF16, tag=f"keff{h}", name=f"keff{h}")
                for h in range(HPC)
            ]

            pending_out = []  # deferred output-projection work items

            def emit_out_group(item, tt, ncx_list):
                jj, yn_ = item
                tb = jj * 512
                for ncx in ncx_list:
                    op = psB.tile([128, 512], F32, tag="b", name="op")
                    for h in range(HPC):
                        nc.tensor.matmul(
                            op[:],
                            yn_[h][:, tt * 128 : (tt + 1) * 128],
                            wo[:, h, ncx * 512 : (ncx + 1) * 512],
                            start=(h == 0),
                            stop=(h == HPC - 1),
                        )
                    osb = sbo.tile([128, 512], F32, tag="osb", name="osb")
                    nc.vector.tensor_copy(osb[:], op[:])
                    nc.gpsimd.dma_start(
                        out_ap[
                            tb + tt * 128 : tb + (tt + 1) * 128,
                            ncx * 512 : (ncx + 1) * 512,
                        ],
                        osb[:],
                    )

            for j in range(NT):
                t0 = j * 512

                # ======== phase 1: xT, kvT, qT for this t-chunk ========
                kvps = []
                qps = []
                xt_sb = [None] * 16

                def do_load(cc):
                    xt = sb6.tile([128, 512], BF16, tag="xt", name="xt")
                    nc.sync.dma_start(
                        xt[:], xt_ap[cc * 128 : (cc + 1) * 128, t0 : t0 + 512]
                    )
                    xt_sb[cc] = xt

                def do_mms(cc):
                    xt = xt_sb[cc]
                    for lc in range(4):
                        nc.tensor.matmul(
                            kvps[lc][:],
                            wdkv_t[cc][:, lc * 128 : (lc + 1) * 128],
                            xt[:],
                            start=(cc == 0),
                            stop=(cc == 15),
                        )
                    for h in range(HPC):
                        nc.tensor.matmul(
                            qps[h][:],
                            wq_t[cc][:, h * S : (h + 1) * S],
                            xt[:],
                            start=(cc == 0),
                            stop=(cc == 15),
                        )

                do_load(0)
                do_load(1)
                do_load(2)
                kvps.extend(
                    psA.tile([128, 512], F32, tag="a", name=f"kvps{i}") for i in range(4)
                )
                qps.extend(
                    psB.tile([128, 512], F32, tag="b", name=f"qps{i}")
                    for i in range(HPC)
                )
                for cc in range(3, 16):
                    do_load(cc)
                    do_mms(cc - 3)
                for cc in range(13, 16):
                    do_mms(cc)

                # evacuate: qT first (frees psB slots for att yps/bps),
                # then kvT (frees psA slots for op/v/keff/aps)
                qt = []
                for h in range(HPC):
                    q = sb5.tile([128, 512], BF16, tag="qT", name="qt")
                    with nc.allow_low_precision(reason="bf16 q"):
                        nc.vector.tensor_copy(q[:], qps[h][:])
                    qt.append(q)
                for lc in range(4):
                    with nc.allow_low_precision(reason="bf16 kv"):
                        nc.vector.tensor_copy(kvT[:, lc, t0 : t0 + 512], kvps[lc][:])

                # ======== attention ========
                nst = 4 * j + 4

                class AttState:
                    pass

                def att_begin(h):
                    st = AttState()
                    st.h = h
                    st.yps = psB.tile([128, 512], F32, tag="b", name="yps")
                    st.bps = psB.tile([128, 512], F32, tag="b", name="bps")
                    st.prev = None  # pending y-matmul item
                    st.pair = None  # ex tile awaiting its pair partner
                    st.pending_den = None  # paired tile awaiting its den matmul
                    st.npair = 0
                    return st

                def y_mm(st, item):
                    i, n0, ex = item
                    nc.tensor.matmul(
                        st.yps[:, n0:512],
                        vsb[:, i, st.h * S : (st.h + 1) * S],
                        ex[:, n0:512],
                        start=(i == 0),
                        stop=(i == nst - 1),
                    )

                def den_mm(st):
                    pr, pidx = st.pending_den
                    nc.tensor.matmul(
                        st.bps[:],
                        allones[:],
                        pr[:],
                        start=(pidx == 0),
                        stop=(pidx == nst // 2 - 1),
                    )
                    st.pending_den = None

                def att_steps(st, i_lo, i_hi):
                    for i in range(i_lo, i_hi):
                        diag = i >= 4 * j
                        n0 = (i - 4 * j) * 128 if diag else 0
                        aps = psA.tile([128, 512], F32, tag="a", name="aps")
                        nc.tensor.matmul(
                            aps[:, n0:512],
                            keff[st.h][:, i * 128 : (i + 1) * 128],
                            qt[st.h][:, n0:512],
                            start=True,
                            stop=True,
                        )
                        if st.prev is not None:
                            y_mm(st, st.prev)
                        if st.pending_den is not None:
                            den_mm(st)
                        ex = sb6.tile([128, 512], BF16, tag="exp", name="ex")
                        nc.scalar.activation(
                            ex[:, n0:512],
                            aps[:, n0:512],
                            mybir.ActivationFunctionType.Exp,
                            scale=SCALE,
                        )
                        if diag:
                            if n0 > 0:
                                nc.vector.memset(ex[:, 0:n0], 0.0)
                            with nc.allow_low_precision(reason="bf16 mask"):
                                nc.vector.tensor_mul(
                                    ex[:, n0 : n0 + 128], ex[:, n0 : n0 + 128], tri[:]
                                )
                        if st.pair is None:
                            st.pair = ex
                        else:
                            pr = sb2.tile([128, 512], BF16, tag="pair", name="pr")
                            with nc.allow_low_precision(reason="bf16 den pair"):
                                nc.vector.tensor_add(pr[:], st.pair[:], ex[:])
                            st.pending_den = (pr, st.npair)
                            st.npair += 1
                            st.pair = None
                        st.prev = (i, n0, ex)

                def att_finish(st):
                    y_mm(st, st.prev)
                    if st.pending_den is not None:
                        den_mm(st)
                    bcs = sb2.tile([128, 512], F32, tag="bcs", name="bcs")
                    nc.vector.reciprocal_approx_fast(bcs[:], st.bps[:])
                    y = sbyn.tile([128, 512], BF16, tag="yn", name="y")
                    with nc.allow_low_precision(reason="bf16 yn"):
                        nc.vector.tensor_mul(y[:], st.yps[:], bcs[:])
                    return y

                def emit_v(tt):
                    vp = psA.tile([128, HPC * S], F32, tag="a", name="vp")
                    for lc in range(4):
                        nc.tensor.matmul(
                            vp[:],
                            kvT[:, lc, t0 + tt * 128 : t0 + (tt + 1) * 128],
                            wuv[:, lc, :],
                            start=(lc == 0),
                            stop=(lc == 3),
                        )
                    with nc.allow_low_precision(reason="bf16 v"):
                        nc.vector.tensor_copy(vsb[:, 4 * j + tt, :], vp[:])

                def emit_keff(h):
                    kp = psA.tile([128, 512], F32, tag="a", name="kp")
                    for lc in range(4):
                        nc.tensor.matmul(
                            kp[:],
                            wukT[:, lc, h * S : (h + 1) * S],
                            kvT[:, lc, t0 : t0 + 512],
                            start=(lc == 0),
                            stop=(lc == 3),
                        )
                    with nc.allow_low_precision(reason="bf16 keff"):
                        nc.vector.tensor_copy(keff[h][:, t0 : t0 + 512], kp[:])

                # ---- schedule: head 0's off-diagonal attention (prior
                # chunks' keff/vsb only) starts right after qt[0]; this
                # chunk's keff/v are computed just in time; the previous
                # chunk's output projection fills exp-latency bubbles ----
                st0 = att_begin(0)
                att_steps(st0, 0, 4 * j)
                emit_keff(0)
                for tt in range(4):
                    emit_v(tt)
                if pending_out:
                    emit_out_group(pending_out[-1], 0, [0, 1])
                att_steps(st0, 4 * j, nst)
                emit_keff(1)
                y0 = att_finish(st0)
                if pending_out:
                    emit_out_group(pending_out[-1], 0, [2, 3])
                    emit_out_group(pending_out[-1], 1, [0, 1])
                st1 = att_begin(1)
                att_steps(st1, 0, nst)
                emit_keff(2)
                y1 = att_finish(st1)
                if pending_out:
                    emit_out_group(pending_out[-1], 1, [2, 3])
                    emit_out_group(pending_out[-1], 2, [0, 1])
                st2 = att_begin(2)
                att_steps(st2, 0, nst)
                emit_keff(3)
                y2 = att_finish(st2)
                if pending_out:
                    emit_out_group(pending_out[-1], 2, [2, 3])
                    emit_out_group(pending_out[-1], 3, [0, 1])
                st3 = att_begin(3)
                att_steps(st3, 0, nst)
                y3 = att_finish(st3)
                if pending_out:
                    emit_out_group(pending_out[-1], 3, [2, 3])
                    pending_out.pop()

                pending_out.append((j, [y0, y1, y2, y3]))

            # flush the last chunk's output projection
            item = pending_out.pop()
            for tt in range(4):
                emit_out_group(item, tt, [0, 1, 2, 3])

    nc.compile()
    return nc


def _get_nc():
    if "nc" not in _CACHE:
        _CACHE["nc"] = _build()
    return _CACHE["nc"]


def kernel(x, w_dkv, w_uk, w_uv, w_q, w_o):
    bf16 = ml_dtypes.bfloat16
    x = np.asarray(x, dtype=np.float32)
    w_dkv = np.asarray(w_dkv, dtype=np.float32).astype(bf16)
    w_uk = np.asarray(w_uk, dtype=np.float32)
    w_uv = np.asarray(w_uv, dtype=np.float32)
    w_q = np.asarray(w_q, dtype=np.float32)
    w_o = np.asarray(w_o, dtype=np.float32)

    nc = _get_nc()

    tri = np.triu(np.ones((128, 128), dtype=np.float32)).astype(bf16)
    allones = np.ones((128, 128), dtype=np.float32).astype(bf16)

    xT = [np.ascontiguousarray(x[b].T).astype(bf16) for b in range(B)]

    in_maps = []
    for c in range(NCORES):
        b = c // 4
        hg = c % 4
        sl = slice(hg * HPC * S, (hg + 1) * HPC * S)
        in_maps.append(
            {
                "xT": xT[b],
                "w_dkv": w_dkv,
                "w_q_sl": np.ascontiguousarray(w_q[:, sl]).astype(bf16),
                "w_ukT_sl": np.ascontiguousarray(w_uk[sl, :].T).astype(bf16),
                "w_uv_sl": np.ascontiguousarray(w_uv[:, sl]).astype(bf16),
                "w_o_sl": np.ascontiguousarray(w_o[sl, :]).astype(bf16),
                "tri": tri,
                "allones": allones,
            }
        )

    kwargs = dict(_CACHE.get("run_kwargs", {}))
    res = bass_utils.run_bass_kernel_spmd(
        nc, in_maps, core_ids=list(range(NCORES)), **kwargs
    )
    _CACHE["last_result"] = res

    out = np.zeros((B, T, C), dtype=np.float64)
    for c in range(NCORES):
        out[c // 4] += res.results[c]["out"]
    return out.astype(np.float32)
